# revision 1
# baseline (speedup 1.0000x reference)
"""Trainium2 Bass kernel for MinEuclideanDistBlockACS (retrieval_knn).

d[n,0,k] = min_{c,w} || x[n,c,w:w+64] - shapelets[c,k,:] ||.

Data-parallel over batch N across 8 cores (8 samples/core = 24 (n,c)-pairs,
no cross-core communication). Per pair, a 65-row bf16 Hankel tile
(64 overlapping x-shift rows DMA'd from DRAM + 1 sliding-||x_w||^2 row)
feeds bf16 PE matmuls with lhsT = [-2*shapelets_c ; 1], producing
t[k,w] = x2[w] - 2<x_w,s_k> in PSUM (4 chunks 3x1024+961, 3-slot ring).

PSUM evacuation is the bottleneck (only DVE+ACT can read PSUM, one PSUM
operand per instruction, fp32-only PSUM on TRN2):
  * ~42% of chunks: DVE tensor_reduce(min) straight to a partials column.
  * ~58%: ACT copy (bf16 cast) into a per-pair collect tile, then DVE
    tensor_scalar(min, accum_out=min) in 4x mode (2-byte packed SBUF).
  * routing pattern cycle ["ADAD","ADAD","AADA"] balances both engines.

x2 rows come from a 4-pair batched PE chain (transpose + ACT square +
prefix matmuls vs [m1|m2]) cast to bf16 and scattered into Hankel row 64
by one SBUF->SBUF DMA per pair. Finale is batched: slot-major partials
(128, 4*24) -> 3 tensor_tensor(min) -> +s2 (c-major replicated) ->
min over channels -> relu -> sqrt -> PE transpose -> out.

Schedule notes (timeline-sim driven): all latency-critical DMAs ride the
SP HWDGE queue in dependency order (x2 write would head-of-line block, so
x2 lives in SBUF); s2rep loads via SWDGE; ACT's first op pre-warms the
Copy/Square/Sqrt table set; per-run ts-accum keeps DVE decoupled from
ACT's in-order stream.
"""

import sys

import numpy as np

for _p in ("/opt/trn_rl_repo",):
    if _p not in sys.path:
        sys.path.insert(0, _p)

import ml_dtypes
import concourse.bass as bass
import concourse.tile as tile
from concourse import bacc, mybir
from concourse.bass_utils import run_bass_kernel_spmd

F32 = mybir.dt.float32
BF16 = mybir.dt.bfloat16
MIN = mybir.AluOpType.min
MAX = mybir.AluOpType.max
ADD = mybir.AluOpType.add
AXX = mybir.AxisListType.X

N, C, L = 64, 3, 4096
K, S = 128, 64
W = L - S + 1  # 4033
NCORES = 8
NPC = N // NCORES
NP = NPC * C  # 24 pairs per core
PAD = 128
BIG = 1.0e4  # > any |t| value; bf16-exact

# chunking: per pair 4 PSUM chunks [1024,1024,1024,961]
CH_OFF = [0, 1024, 2048, 3072]
CH_LEN = [1024, 1024, 1024, 961]

# evac routing: per-pair pattern of chunk->engine ('D'=DVE reduce,'A'=ACT copy)
# global ratio target ~42% DVE / 58% ACT (incl. DVE ts4x second stage).
# Every pair needs >=1 'A' (ts4x slot always written) - guaranteed below.
PATTERNS = [
    "ADAD", "ADAD", "AADA",
]

_CACHE = {}


def _build_bass():
    nc = bacc.Bacc("TRN2", target_bir_lowering=False, debug=False)

    x16_d = nc.dram_tensor("x16", (NP * L + PAD,), BF16, kind="ExternalInput")
    x32_d = nc.dram_tensor("x32", (NP * L + PAD,), F32, kind="ExternalInput")
    lhsT_d = nc.dram_tensor("lhsT", (S + 1, C * K), BF16, kind="ExternalInput")
    m1m2_d = nc.dram_tensor("m1m2", (S, 2 * S), BF16, kind="ExternalInput")
    ident_d = nc.dram_tensor("ident", (K, K), F32, kind="ExternalInput")
    s2rep_d = nc.dram_tensor("s2rep", (K, NP), F32, kind="ExternalInput")
    out_d = nc.dram_tensor("out", (K, NPC), F32, kind="ExternalOutput")

    with tile.TileContext(nc) as tc:
        with (
            tc.tile_pool(name="consts", bufs=1) as consts,
            tc.tile_pool(name="hankp", bufs=4) as hankp,
            tc.tile_pool(name="colp", bufs=4) as colp,
            tc.tile_pool(name="smallp", bufs=2) as smallp,
            tc.tile_pool(name="onep", bufs=1) as onep,
            tc.tile_pool(name="psp", bufs=1, space="PSUM") as psp,
        ):
            # ---- lead-in constant loads (scalar queue; hankels ride sync)
            ident_sb = consts.tile([K, K], F32)
            nc.sync.dma_start(ident_sb[:, :], ident_d[:, :])
            ident16_sb = consts.tile([K, K], BF16)
            nc.vector.tensor_copy(ident16_sb[:, :], ident_sb[:, :])
            m1m2_sb = consts.tile([S, 2 * S], BF16)
            lhsT_sb = consts.tile([S + 1, C * K], BF16)
            s2rep_sb = consts.tile([K, NP], F32)
            nc.gpsimd.dma_start(s2rep_sb[:, :], s2rep_d[:, :])

            # partials: slot-major (128, 4*NP): col = slot*NP + pairidx_cmajor
            partials = onep.tile([K, 4 * NP], F32)
            nc.vector.memset(partials[:, :], BIG)
            junk16a = onep.tile([K, 4096], BF16)
            junk16b = onep.tile([K, 4096], BF16)
            warm = onep.tile([K, 1], F32)
            nc.scalar.activation(warm[:, :], ident_sb[:, 0:1],
                                 mybir.ActivationFunctionType.Square)

            # ---- x2 chain for 4 pairs (j covers pairs 4j..4j+3)
            def chain4(j):
                scrT = psp.tile([S, 2 * K], BF16, tag="scratchT", bufs=1)
                scr = psp.tile([K, 384], F32, tag="scratch", bufs=1)
                xsqT = smallp.tile([S, 2 * K], BF16, tag="xsqT")
                # one merged bf16 compact load for all 4 pairs
                compact = smallp.tile([K, 2 * S], BF16, tag="compact")
                nc.sync.dma_start(
                    compact[:, :],
                    bass.AP(tensor=x16_d[:].tensor, offset=4 * j * L,
                            ap=[[S, 128], [2 * L, 2], [1, S]]),
                )
                if j == 0:
                    # const loads ride the queue behind chain0's compact
                    nc.sync.dma_start(lhsT_sb[:, :], lhsT_d[:, :])
                    nc.sync.dma_start(m1m2_sb[:, :], m1m2_d[:, :])
                for h in range(2):  # h=0: pairs 4j,4j+1 ; h=1: pairs 4j+2,4j+3
                    nc.tensor.transpose(
                        scrT[:, 128 * h:128 * h + 128],
                        compact[:, S * h:S * h + S],
                        ident16_sb[:, :])
                # one batched square for all 4 pairs
                nc.scalar.activation(
                    xsqT[:, :], scrT[:, :],
                    mybir.ActivationFunctionType.Square)
                # prefix matmuls into scr cols [256:384]: h half -> cols 64h
                for h in range(2):
                    x2ps = scr[:, 256 + 64 * h:256 + 64 * h + 64]
                    lh = xsqT[:, 128 * h:128 * h + 128]
                    nc.tensor.matmul(x2ps, lh, m1m2_sb[:, 0:S],
                                     start=True, stop=False)
                    nc.tensor.matmul(
                        scr[0:63, 256 + 64 * h:256 + 64 * h + 64],
                        xsqT[:, 128 * h + 1:128 * h + 64],
                        m1m2_sb[:, S:2 * S],
                        start=False, stop=False, skip_group_check=True)
                    nc.tensor.matmul(
                        scr[64:127, 256 + 64 * h:256 + 64 * h + 64],
                        xsqT[:, 128 * h + 65:128 * h + 128],
                        m1m2_sb[:, S:2 * S],
                        start=False, stop=(h == 1), skip_group_check=True)
                x2sb = smallp.tile([K, K], BF16, tag="x2sb", bufs=4)
                nc.scalar.copy(x2sb[:, :], scr[:, 256:384])
                x2tiles[j] = x2sb

            def hankel_rows(p):
                hank = hankp.tile([S + 1, L], BF16, tag="hank")
                nc.sync.dma_start(
                    hank[0:S, :],
                    bass.AP(tensor=x16_d[:].tensor, offset=p * L,
                            ap=[[1, S], [1, L]]),
                )
                return hank

            def x2row(p):
                # x2 row for pair p from chain tile j=p//4:
                # x2sb[64*(p%4 in {0,1} -> row half) ...]: src slice (64, 64)
                j, r = divmod(p, 4)
                half, sub = divmod(r, 2)
                x2sb = x2tiles[j]
                nc.sync.dma_start(
                    live_hank[p][S:S + 1, 0:L],
                    x2sb[64 * sub:64 * sub + 64, 64 * half:64 * half + 64],
                )

            live_hank = {}
            x2tiles = {}

            def main(p):
                n, c = divmod(p, C)
                cm = c * NPC + n  # c-major pair column index
                hank = live_hank.pop(p)
                lhsT_c = lhsT_sb[:, c * K:(c + 1) * K]
                pat = PATTERNS[p % len(PATTERNS)]
                collect = colp.tile([K, 4096], BF16, tag="collect")
                runs = []  # contiguous A runs: list of (beg, end, first_ch)
                for ch in range(4):
                    w0, wl = CH_OFF[ch], CH_LEN[ch]
                    mps = psp.tile([K, 1024], F32, tag="chunk", bufs=3)
                    nc.tensor.matmul(mps[:, 0:512], lhsT_c,
                                     hank[:, w0:w0 + 512],
                                     start=True, stop=True)
                    nc.tensor.matmul(mps[:, 512:wl], lhsT_c,
                                     hank[:, w0 + 512:w0 + wl],
                                     start=True, stop=True)
                    if pat[ch] == "D":
                        nc.vector.tensor_reduce(
                            partials[:, ch * NP + cm:ch * NP + cm + 1],
                            mps[:, 0:wl], axis=AXX, op=MIN)
                    else:
                        nc.scalar.copy(collect[:, w0:w0 + wl], mps[:, 0:wl])
                        if runs and runs[-1][1] == w0:
                            runs[-1] = (runs[-1][0], w0 + wl, runs[-1][2])
                        else:
                            runs.append((w0, w0 + wl, ch))
                junk16 = junk16a if p % 2 == 0 else junk16b
                for beg, end, ch0 in runs:
                    nc.vector.tensor_scalar(
                        junk16[:, 0:end - beg], collect[:, beg:end], BIG, None,
                        op0=MIN, op1=MIN,
                        accum_out=partials[:, ch0 * NP + cm:ch0 * NP + cm + 1])

            PIPE = 3
            NCH = (NP + 3) // 4
            chain4(0)
            for step in range(NP + PIPE):
                if step < NP:
                    live_hank[step] = hankel_rows(step)
                    if step % 4 == 0 and step // 4 + 1 < NCH:
                        chain4(step // 4 + 1)
                    x2row(step)
                if step >= PIPE:
                    main(step - PIPE)

            # ---- batched finale
            m1t = onep.tile([K, 2 * NP], F32)
            nc.vector.tensor_tensor(m1t[:, 0:NP], partials[:, 0:NP],
                                    partials[:, NP:2 * NP], op=MIN)
            nc.vector.tensor_tensor(m1t[:, NP:2 * NP], partials[:, 2 * NP:3 * NP],
                                    partials[:, 3 * NP:4 * NP], op=MIN)
            chanmin = onep.tile([K, NP], F32)
            nc.vector.tensor_tensor(chanmin[:, :], m1t[:, 0:NP],
                                    m1t[:, NP:2 * NP], op=MIN)
            # + s2 (c-major replicated), min over channels, relu
            d2 = onep.tile([K, NP], F32)
            nc.vector.tensor_tensor(d2[:, :], chanmin[:, :], s2rep_sb[:, :],
                                    op=ADD)
            dmin = onep.tile([K, NPC], F32)
            nc.vector.tensor_tensor(dmin[:, :], d2[:, 0:NPC], d2[:, NPC:2 * NPC],
                                    op=MIN)
            nc.vector.tensor_tensor(dmin[:, :], dmin[:, :], d2[:, 2 * NPC:3 * NPC],
                                    op=MIN)
            dr = onep.tile([K, NPC], F32)
            nc.vector.tensor_scalar(dr[:, :], dmin[:, :], 0.0, None, op0=MAX)
            outT = onep.tile([K, NPC], F32)
            nc.scalar.sqrt(outT[:, :], dr[:, :])
            # store (K, NPC) column-major; host transposes the 8x128 result
            nc.sync.dma_start(out_d[:, :], outT[:, :])

    nc.finalize()
    return nc


def _host_consts(shapelets: np.ndarray):
    shp = np.asarray(shapelets, np.float32)
    lhsT = np.zeros((S + 1, C * K), np.float32)
    for c in range(C):
        lhsT[:S, c * K:(c + 1) * K] = -2.0 * shp[c].T
        lhsT[S, c * K:(c + 1) * K] = 1.0
    s2 = (shp * shp).sum(-1)  # (C, K)
    s2rep = np.zeros((K, NP), np.float32)
    for c in range(C):
        for n in range(NPC):
            s2rep[:, c * NPC + n] = s2[c]
    r = np.arange(S)
    m1 = (r[:, None] >= r[None, :]).astype(np.float32)
    m2 = (r[:, None] < r[None, :]).astype(np.float32)
    m1m2 = np.concatenate([m1, m2], axis=1)
    ident = np.eye(K, dtype=np.float32)
    return lhsT, s2rep, m1m2, ident


def kernel(x: np.ndarray, shapelets: np.ndarray, _trace: bool = False):
    x = np.asarray(x, np.float32)
    lhsT, s2rep, m1m2, ident = _host_consts(shapelets)

    if "nc" not in _CACHE:
        _CACHE["nc"] = _build_bass()
    nc = _CACHE["nc"]

    bf = lambda a: np.ascontiguousarray(a).astype(ml_dtypes.bfloat16)
    in_maps = []
    for core in range(NCORES):
        shard = x[core * NPC:(core + 1) * NPC].ravel()
        x32 = np.concatenate([shard, np.zeros(PAD, np.float32)])
        in_maps.append({
            "x16": bf(x32), "x32": x32, "lhsT": bf(lhsT),
            "m1m2": bf(m1m2), "ident": ident, "s2rep": s2rep,
        })

    res = run_bass_kernel_spmd(nc, in_maps, core_ids=list(range(NCORES)),
                               trace=_trace)
    _CACHE["last_result"] = res
    out = np.concatenate([res.results[i]["out"].T for i in range(NCORES)], axis=0)
    return out.reshape(N, 1, K).astype(np.float32)

